# revision 12
# baseline (speedup 1.0000x reference)
"""MDTA (Restormer transposed-channel attention) Trainium2 Bass kernel.

Data-parallel over batch: 8 images -> 8 NeuronCores, one image per core.

Per-core pipeline (image = [192, 128, 128] f32):
  1. GEMM1 (1x1 conv): qkv = w_qkv @ x  on PE, bf16 in / f32 psum,
     written into a row-padded buffer (row stride 130: [pad, 128 data, pad])
     so the depthwise conv reads zeros at image edges.
  2. Depthwise 3x3: 9 accumulating PE matmuls per tile with diagonal
     stationaries diag(w_tap[ch]); the tap shift is the rhs AP offset
     (130*dy + dx) into the padded buffer; the sum lands in PSUM.
  3. Transposed attention, processed in 8 chunks of 16 image rows:
     per chunk transpose q,k (PE) and accumulate logits qT.T@kT into a
     persistent PSUM bank; L2-norm sums via ACT Square+accum_out.
     After all chunks: scale logits by rsqrt(row)/rsqrt(col) (+temperature),
     per-head softmax on the 5 diagonal block pieces, attn@v + projection.
"""

import os
import sys
import numpy as np

for _p in ("/opt/trn_rl_repo",):
    if _p not in sys.path and os.path.isdir(_p):
        sys.path.insert(0, _p)

import ml_dtypes

BF16 = ml_dtypes.bfloat16

B, C, H, W = 8, 192, 128, 128
HEADS, DH = 4, 48
C3 = 3 * C            # 576
HW = H * W            # 16384
RT = W + 2            # padded row stride = 130 (col0 pad, 1..128 data, 129 pad)
TY = 16               # image rows per chunk
NCHUNK = H // TY      # 8
SLOTS = TY + 2        # 18 row slots per chunk (halo)
# partition tiles over 576 channels
PT5 = [128, 128, 128, 128, 64]
PO5 = [0, 128, 256, 384, 512]
# partition tiles over 192
PT2 = [128, 64]
PO2 = [0, 128]

_CACHE = {}


def _build():
    import concourse.bass as bass
    import concourse.bacc as bacc
    import concourse.tile as tile
    import concourse.mybir as mybir

    f32 = mybir.dt.float32
    bf16 = mybir.dt.bfloat16
    Alu = mybir.AluOpType
    Act = mybir.ActivationFunctionType

    nc = bacc.Bacc("TRN2", target_bir_lowering=False, debug=False,
                   enable_asserts=False)

    x_d = nc.dram_tensor("x0", [C, H, W], bf16, kind="ExternalInput").ap()
    wq_d = nc.dram_tensor("wq", [C, C3], bf16, kind="ExternalInput").ap()
    wdg_d = nc.dram_tensor("wdg", [128, 45 * 128], bf16, kind="ExternalInput").ap()
    wp_d = nc.dram_tensor("wp", [C, C], bf16, kind="ExternalInput").ap()
    tv_d = nc.dram_tensor("tv", [C, 1], f32, kind="ExternalInput").ap()
    idb_d = nc.dram_tensor("idb", [128, 128], bf16, kind="ExternalInput").ap()
    idf_d = nc.dram_tensor("idf", [128, 128], f32, kind="ExternalInput").ap()
    ones_d = nc.dram_tensor("ones1", [1, 128], f32, kind="ExternalInput").ap()
    mka_d = nc.dram_tensor("mka", [128, C], f32, kind="ExternalInput").ap()
    mkb_d = nc.dram_tensor("mkb", [64, C], f32, kind="ExternalInput").ap()
    out_d = nc.dram_tensor("out0", [C, H, W], f32, kind="ExternalOutput").ap()

    with tile.TileContext(nc) as tc:
        with (
            tc.tile_pool(name="const", bufs=1) as cpool,
            tc.tile_pool(name="big", bufs=1) as big,
            tc.tile_pool(name="xin", bufs=2) as xpool,
            tc.tile_pool(name="qk", bufs=1) as qkpool,
            tc.tile_pool(name="qk2", bufs=2) as qkpool2,
            tc.tile_pool(name="small", bufs=1) as small,
        ):
            # ---- constants into SBUF ----
            wq_a = cpool.tile([128, C3], bf16, tag="wqa")
            wq_b = cpool.tile([64, C3], bf16, tag="wqb")
            wdg = cpool.tile([128, 45, 128], bf16, tag="wdg")
            wp_a = cpool.tile([128, C], bf16, tag="wpa")
            wp_b = cpool.tile([64, C], bf16, tag="wpb")
            idb = cpool.tile([128, 128], bf16, tag="idb")
            idf = cpool.tile([128, 128], f32, tag="idf")
            ones1 = cpool.tile([1, 128], f32, tag="ones1")
            tv_a = cpool.tile([128, 1], f32, tag="tva")
            tv_b = cpool.tile([64, 1], f32, tag="tvb")
            # order: gemm1 operands first (wq, then chunk-0 x below), then
            # the depthwise weights, then everything only needed later
            nc.sync.dma_start(wq_a[:], wq_d[0:128, :])
            nc.sync.dma_start(wq_b[:], wq_d[128:192, :])
            # prefetch chunk 0's x slab ahead of the remaining constants
            xa0 = xpool.tile([128, SLOTS, W], bf16, tag="xa")
            xb0 = xpool.tile([64, SLOTS, W], bf16, tag="xb")
            nc.vector.memset(xa0[:, 0, :], 0.0)
            nc.vector.memset(xb0[:, 0, :], 0.0)
            nc.sync.dma_start(xa0[:, 1:SLOTS, :], x_d[0:128, 0:SLOTS - 1, :])
            nc.sync.dma_start(xb0[:, 1:SLOTS, :], x_d[128:192, 0:SLOTS - 1, :])
            nc.sync.dma_start(wdg[:], wdg_d[:, :])
            nc.sync.dma_start(idb[:], idb_d[:])
            nc.sync.dma_start(wp_a[:], wp_d[0:128, :])
            nc.sync.dma_start(wp_b[:], wp_d[128:192, :])
            nc.sync.dma_start(idf[:], idf_d[:])
            nc.sync.dma_start(ones1[:], ones_d[:])
            nc.sync.dma_start(tv_a[:], tv_d[0:128, :])
            nc.sync.dma_start(tv_b[:], tv_d[128:192, :])
            mk_a = cpool.tile([128, C], f32, tag="mka")
            mk_b = cpool.tile([64, C], f32, tag="mkb")
            nc.sync.dma_start(mk_a[:], mka_d[:])
            nc.sync.dma_start(mk_b[:], mkb_d[:])

            # ---- persistent big buffers ----
            # padded qkv (post 1x1 conv), f32: [128p, 5 ch-tiles, 18 slots, 130]
            qkv = big.tile([128, 5, SLOTS, RT], bf16, tag="qkv")
            # only the pad columns (0 and RT-1) need zeroing; the data
            # region is overwritten every chunk and the halo slots get
            # zeros via the zeroed x edge rows.
            nc.vector.memset(qkv[:, :, :, 0:1], 0.0)
            nc.vector.memset(qkv[:, :, :, RT - 1:RT], 0.0)
            # v in bf16 for pass 2: [128p, 128 row-groups of 128, 128]
            v_a = big.tile([128, H, W], bf16, tag="va")
            v_b = big.tile([64, H, W], bf16, tag="vb")
            # l2 norm partial sums per chunk
            qss_a = small.tile([128, NCHUNK], f32, tag="qssa")
            qss_b = small.tile([64, NCHUNK], f32, tag="qssb")
            kss_a = small.tile([128, NCHUNK], f32, tag="kssa")
            kss_b = small.tile([64, NCHUNK], f32, tag="kssb")

            with (
                tc.tile_pool(name="psg", bufs=2, space="PSUM") as psg,
                tc.tile_pool(name="psd", bufs=2, space="PSUM") as psd,
                tc.tile_pool(name="pst", bufs=2, space="PSUM") as pst,
                tc.tile_pool(name="psl", bufs=1, space="PSUM") as psl,
            ):
                lg_a = psl.tile([128, C], f32, tag="lga")
                lg_b = psl.tile([64, C], f32, tag="lgb")

                for ch in range(NCHUNK):
                    y0 = ch * TY
                    # ---- load x chunk (rows y0-1 .. y0+16) ----
                    if ch == 0:
                        xa, xb = xa0, xb0
                    else:
                        xa = xpool.tile([128, SLOTS, W], bf16, tag="xa")
                        xb = xpool.tile([64, SLOTS, W], bf16, tag="xb")
                        lo = y0 - 1
                        hi = min(y0 + TY, H - 1)
                        s1 = hi - (y0 - 1)
                        if ch == NCHUNK - 1:
                            nc.vector.memset(xa[:, SLOTS - 1, :], 0.0)
                            nc.vector.memset(xb[:, SLOTS - 1, :], 0.0)
                        nc.sync.dma_start(xa[:, 0:s1 + 1, :], x_d[0:128, lo:hi + 1, :])
                        nc.sync.dma_start(xb[:, 0:s1 + 1, :], x_d[128:192, lo:hi + 1, :])

                    # ---- GEMM1: qkv = wq.T @ x, 3 rows (384 cols) at a time ----
                    for r in range(5):
                        po, pn = PO5[r], PT5[r]
                        for j in range(SLOTS // 3):
                            pg = psg.tile([128, 3, W], f32, tag="pg")
                            nc.tensor.matmul(
                                pg[0:pn, :, :], wq_a[:, po:po + pn],
                                xa[:, 3 * j:3 * j + 3, :], start=True, stop=False)
                            nc.tensor.matmul(
                                pg[0:pn, :, :], wq_b[:, po:po + pn],
                                xb[:, 3 * j:3 * j + 3, :], start=False, stop=True)
                            nc.any.tensor_copy(
                                qkv[0:pn, r, 3 * j:3 * j + 3, 1:1 + W],
                                pg[0:pn, :, :])

                    # ---- depthwise 3x3 + split/copy q,k,v, interleaved with
                    # the q/k transposes and logit matmuls so the PE always
                    # has HAM-visible (regular matmul) work in flight ----
                    q_a = qkpool2.tile([128, TY, W], bf16, tag="qa")
                    q_b = qkpool2.tile([64, TY, W], bf16, tag="qb")
                    k_a = qkpool2.tile([128, TY, W], bf16, tag="ka")
                    k_b = qkpool2.tile([64, TY, W], bf16, tag="kb")
                    qT = qkpool.tile([128, TY, C], bf16, tag="qT")
                    kT = qkpool.tile([128, TY, C], bf16, tag="kT")

                    def dw(r, t):
                        po, pn = PO5[r], PT5[r]
                        pd = psd.tile([128, 4, W], f32, tag="pd")
                        for kk in range(9):
                            dy, dx = kk // 3 - 1, kk % 3 - 1
                            srow = 1 + 4 * t + dy
                            nc.tensor.matmul(
                                pd[0:pn, :, :],
                                wdg[0:pn, r * 9 + kk, 0:pn],
                                qkv[0:pn, r, srow:srow + 4, 1 + dx:1 + dx + W],
                                start=(kk == 0), stop=(kk == 8))
                        dst = 4 * t
                        if r == 0:
                            nc.any.tensor_copy(q_a[:, dst:dst + 4, :], pd[:, :, :])
                        elif r == 1:
                            nc.any.tensor_copy(q_b[0:64, dst:dst + 4, :], pd[0:64, :, :])
                            nc.any.tensor_copy(k_a[0:64, dst:dst + 4, :], pd[64:128, :, :])
                        elif r == 2:
                            nc.any.tensor_copy(k_a[64:128, dst:dst + 4, :], pd[0:64, :, :])
                            nc.any.tensor_copy(k_b[0:64, dst:dst + 4, :], pd[64:128, :, :])
                        elif r == 3:
                            nc.any.tensor_copy(v_a[:, y0 + dst:y0 + dst + 4, :], pd[:, :, :])
                        else:
                            nc.any.tensor_copy(v_b[0:64, y0 + dst:y0 + dst + 4, :], pd[0:64, :, :])

                    def tr(s):
                        # transpose one slot of q and k into qT/kT (PSUM hop)
                        tpq = pst.tile([128, C], bf16, tag="tp")
                        nc.tensor.transpose(tpq[:, 0:128], q_a[:, s, :], idb[:])
                        nc.tensor.transpose(tpq[:, 128:192], q_b[:, s, :], idb[0:64, 0:64])
                        nc.any.tensor_copy(qT[:, s, :], tpq[:, :])
                        tpk = pst.tile([128, C], bf16, tag="tp")
                        nc.tensor.transpose(tpk[:, 0:128], k_a[:, s, :], idb[:])
                        nc.tensor.transpose(tpk[:, 128:192], k_b[:, s, :], idb[0:64, 0:64])
                        nc.any.tensor_copy(kT[:, s, :], tpk[:, :])

                    def lg(s):
                        first = (ch == 0 and s == 0)
                        last = (ch == NCHUNK - 1 and s == TY - 1)
                        nc.tensor.matmul(lg_a[:, :], qT[:, s, 0:128], kT[:, s, :],
                                         start=first, stop=last, skip_group_check=True)
                        nc.tensor.matmul(lg_b[:, :], qT[:, s, 128:192], kT[:, s, :],
                                         start=first, stop=last, skip_group_check=True)

                    # interleave so no PE transpose bunch exceeds ~1us of
                    # HAM-invisible work between regular matmul stretches
                    for t in range(TY // 4):
                        dw(0, t)
                        dw(1, t)
                        dw(2, t)
                        tr(4 * t)
                        dw(3, t)
                        tr(4 * t + 1)
                        dw(4, t)
                        tr(4 * t + 2)
                        lg(4 * t)
                        tr(4 * t + 3)
                        lg(4 * t + 1)
                        lg(4 * t + 2)
                        lg(4 * t + 3)

                    # ---- L2 norm partial sums (ACT: square + accum) ----
                    sqs = qkpool.tile([128, TY, W], bf16, tag="sqs")
                    nc.scalar.activation(sqs[:, :, :], q_a[:, :, :], Act.Square,
                                         accum_out=qss_a[:, ch:ch + 1])
                    nc.scalar.activation(sqs[0:64, :, :], q_b[:, :, :], Act.Square,
                                         accum_out=qss_b[:, ch:ch + 1])
                    nc.scalar.activation(sqs[:, :, :], k_a[:, :, :], Act.Square,
                                         accum_out=kss_a[:, ch:ch + 1])
                    nc.scalar.activation(sqs[0:64, :, :], k_b[:, :, :], Act.Square,
                                         accum_out=kss_b[:, ch:ch + 1])

                # ================= softmax =================
                # rsqrt of row sums (+ temperature on q side)
                rq_a = small.tile([128, 1], f32, tag="rqa")
                rq_b = small.tile([64, 1], f32, tag="rqb")
                rk_a = small.tile([128, 1], f32, tag="rka")
                rk_b = small.tile([64, 1], f32, tag="rkb")
                tmp_a = small.tile([128, 1], f32, tag="tmpa")
                tmp_b = small.tile([64, 1], f32, tag="tmpb")
                for (ss, rr, tmp, tvx) in ((qss_a, rq_a, tmp_a, tv_a),
                                           (qss_b, rq_b, tmp_b, tv_b),
                                           (kss_a, rk_a, tmp_a, None),
                                           (kss_b, rk_b, tmp_b, None)):
                    nc.vector.tensor_reduce(tmp[:], ss[:], mybir.AxisListType.X, Alu.add)
                    nc.scalar.activation(tmp[:], tmp[:], Act.Sqrt)
                    nc.vector.tensor_scalar_max(tmp[:], tmp[:], 1e-12)
                    nc.vector.reciprocal(rr[:], tmp[:])
                    if tvx is not None:
                        nc.vector.tensor_tensor(rr[:], rr[:], tvx[:], Alu.mult)

                # copy logits out of psum, scale rows by rq
                L_a = small.tile([128, C], f32, tag="La")
                L_b = small.tile([64, C], f32, tag="Lb")
                nc.vector.tensor_scalar(L_a[:], lg_a[:], rq_a[:], None, Alu.mult)
                nc.vector.tensor_scalar(L_b[:], lg_b[:], rq_b[:], None, Alu.mult)

            with tc.tile_pool(name="psx", bufs=1, space="PSUM") as psx:
                # warm-keeper: dummy matmuls that read the freshly-written
                # softmax intermediates.  Each is data-dependent on the
                # producing op, so they spread across the softmax phase and
                # keep the PE clock-gate (HAM) from re-throttling before
                # pass 2 starts.
                warm = psx.tile([128, C], f32, tag="warm")

                def keep_warm(src, np_=128):
                    nc.tensor.matmul(warm[0:np_, :], idf[0:np_, 0:np_], src,
                                     start=True, stop=True)

                # column scale: bcast rk over partitions via K=1 matmul
                rkrow = small.tile([1, C], f32, tag="rkrow")
                pb = psx.tile([128, C], f32, tag="pb")
                nc.tensor.transpose(pb[0:1, 0:128], rk_a[:], idf[:])
                nc.tensor.transpose(pb[0:1, 128:192], rk_b[:], idf[0:64, 0:64])
                nc.any.tensor_copy(rkrow[:], pb[0:1, 0:192])
                pbc = psx.tile([128, C], f32, tag="pbc")
                nc.tensor.matmul(pbc[:, :], ones1[:], rkrow[:], start=True, stop=True)
                keep_warm(L_a[:, :])
                nc.vector.tensor_tensor(L_a[:], L_a[:], pbc[:, :], Alu.mult)
                nc.vector.tensor_tensor(L_b[:], L_b[:], pbc[0:64, :], Alu.mult)
                keep_warm(L_a[:, :])

                # full-row softmax; cross-head blocks masked to -1e30 -> exp 0
                attn_a = small.tile([128, C], bf16, tag="atta")
                attn_b = small.tile([64, C], bf16, tag="attb")
                mx = small.tile([128, 1], f32, tag="mx")
                sm = small.tile([128, 1], f32, tag="sm")
                E = small.tile([128, C], f32, tag="E")
                for (L, A, mk, np_) in ((L_a, attn_a, mk_a, 128),
                                        (L_b, attn_b, mk_b, 64)):
                    nc.vector.tensor_tensor(L[:], L[:], mk[:], Alu.add)
                    keep_warm(L[:, :], np_)
                    nc.vector.tensor_reduce(mx[0:np_, :], L[:],
                                            mybir.AxisListType.X, Alu.max)
                    nc.vector.tensor_scalar_mul(mx[0:np_, :], mx[0:np_, :], -1.0)
                    nc.scalar.activation(E[0:np_, :], L[:], Act.Exp,
                                         bias=mx[0:np_, :],
                                         accum_out=sm[0:np_, :])
                    keep_warm(E[0:np_, :], np_)
                    nc.vector.reciprocal(sm[0:np_, :], sm[0:np_, :])
                    nc.scalar.activation(A[:], E[0:np_, :], Act.Copy,
                                         scale=sm[0:np_, :])
                    keep_warm(E[0:np_, :], np_)

                # MT[d, o] = sum_c attn[c, d] * wp[c, o]  ((Wp @ attn)
                # transposed) -- folds the projection into a tiny GEMM so
                # pass 2 is a single stream over v.
                MT_a = small.tile([128, C], bf16, tag="MTa")
                MT_b = small.tile([64, C], bf16, tag="MTb")
                pMa = psx.tile([128, C], f32, tag="pMa")
                pMb = psx.tile([64, C], f32, tag="pMb")
                nc.tensor.matmul(pMa[:, :], attn_a[:, 0:128], wp_a[:, :],
                                 start=True, stop=False)
                nc.tensor.matmul(pMa[:, :], attn_b[:, 0:128], wp_b[:, :],
                                 start=False, stop=True)
                nc.tensor.matmul(pMb[:, :], attn_a[:, 128:192], wp_a[:, :],
                                 start=True, stop=False)
                nc.tensor.matmul(pMb[:, :], attn_b[:, 128:192], wp_b[:, :],
                                 start=False, stop=True)
                nc.any.tensor_copy(MT_a[:, :], pMa[:, :])
                nc.any.tensor_copy(MT_b[:, :], pMb[:, :])

            # ---- pass 2: out = MT.T @ v ----
            with (
                tc.tile_pool(name="ps2", bufs=4, space="PSUM") as ps2,
                tc.tile_pool(name="o2", bufs=3) as opool,
            ):
                    for g in range(HW // 512):
                        r4 = 4 * g
                        poa = ps2.tile([128, 4, W], f32, tag="poa")
                        pob = ps2.tile([64, 4, W], f32, tag="pob")
                        nc.tensor.matmul(poa[:, :, :], MT_a[:, 0:128],
                                         v_a[:, r4:r4 + 4, :], start=True, stop=False)
                        nc.tensor.matmul(poa[:, :, :], MT_b[:, 0:128],
                                         v_b[:, r4:r4 + 4, :], start=False, stop=True)
                        nc.tensor.matmul(pob[:, :, :], MT_a[:, 128:192],
                                         v_a[:, r4:r4 + 4, :], start=True, stop=False)
                        nc.tensor.matmul(pob[:, :, :], MT_b[:, 128:192],
                                         v_b[:, r4:r4 + 4, :], start=False, stop=True)
                        ot_a = opool.tile([128, 4, W], f32, tag="ota")
                        ot_b = opool.tile([64, 4, W], f32, tag="otb")
                        nc.vector.tensor_copy(ot_a[:], poa[:, :, :])
                        nc.any.tensor_copy(ot_b[:], pob[:, :, :])
                        nc.sync.dma_start(out_d[0:128, r4:r4 + 4, :], ot_a[:])
                        nc.sync.dma_start(out_d[128:192, r4:r4 + 4, :], ot_b[:])

    nc.compile()
    return nc


def _prep_weights(w_qkv, w_dw, w_project, temperature):
    wq = np.ascontiguousarray(w_qkv.T).astype(BF16)              # [192, 576]
    wp = np.ascontiguousarray(w_project.T).astype(BF16)          # [192, 192]
    # diagonal stationaries: [128, 45*128] f32, block (r*9+k) = diag(w_dw[ch, k])
    wdg = np.zeros((128, 45, 128), np.float32)
    wd = w_dw.reshape(C3, 9)
    for r in range(5):
        po, pn = PO5[r], PT5[r]
        for k in range(9):
            blk = wdg[:, r * 9 + k, :]
            blk[np.arange(pn), np.arange(pn)] = wd[po:po + pn, k]
    tv = np.repeat(temperature.reshape(HEADS), DH).reshape(C, 1).astype(np.float32)
    mk = np.full((C, C), -1e30, np.float32)
    for h in range(HEADS):
        mk[h * DH:(h + 1) * DH, h * DH:(h + 1) * DH] = 0.0
    return {
        "wq": wq,
        "wp": wp,
        "wdg": wdg.reshape(128, 45 * 128).astype(BF16),
        "tv": tv,
        "idb": np.eye(128, dtype=BF16),
        "idf": np.eye(128, dtype=np.float32),
        "ones1": np.ones((1, 128), np.float32),
        "mka": mk[0:128],
        "mkb": mk[128:192],
    }


def kernel(x, w_qkv, w_dw, w_project, temperature, heads):
    from concourse import bass_utils

    x = np.asarray(x, np.float32)
    key = "nc"
    if key not in _CACHE:
        _CACHE[key] = _build()
    nc = _CACHE[key]

    shared = _prep_weights(np.asarray(w_qkv, np.float32),
                           np.asarray(w_dw, np.float32),
                           np.asarray(w_project, np.float32),
                           np.asarray(temperature, np.float32))
    in_maps = []
    for i in range(B):
        m = dict(shared)
        m["x0"] = x[i].reshape(C, H, W).astype(BF16)
        in_maps.append(m)

    res = bass_utils.run_bass_kernel_spmd(nc, in_maps, core_ids=list(range(B)))
    outs = [r["out0"].reshape(C, H, W) for r in res.results]
    return np.stack(outs, axis=0).astype(np.float32)


if __name__ == "__main__":
    rng = np.random.default_rng(0)
    x = rng.standard_normal((B, C, H, W), np.float32)
    w_qkv = rng.standard_normal((C3, C), np.float32) / np.sqrt(C)
    w_dw = rng.standard_normal((C3, 1, 3, 3), np.float32) / 3.0
    w_project = rng.standard_normal((C, C), np.float32) / np.sqrt(C)
    temperature = np.ones((HEADS, 1, 1), np.float32)
    y = kernel(x=x, w_qkv=w_qkv, w_dw=w_dw, w_project=w_project,
               temperature=temperature, heads=HEADS)
    print(y.shape, y.dtype)



# revision 15
# speedup vs baseline: 1.0388x; 1.0388x over previous
"""MDTA (Restormer transposed-channel attention) Trainium2 Bass kernel.

Data-parallel over batch: 8 images -> 8 NeuronCores, one image per core.

Per-core pipeline (image = [192, 128, 128] f32):
  1. GEMM1 (1x1 conv): qkv = w_qkv @ x  on PE, bf16 in / f32 psum,
     written into a row-padded buffer (row stride 130: [pad, 128 data, pad])
     so the depthwise conv reads zeros at image edges.
  2. Depthwise 3x3: 9 accumulating PE matmuls per tile with diagonal
     stationaries diag(w_tap[ch]); the tap shift is the rhs AP offset
     (130*dy + dx) into the padded buffer; the sum lands in PSUM.
  3. Transposed attention, processed in 8 chunks of 16 image rows:
     per chunk transpose q,k (PE) and accumulate logits qT.T@kT into a
     persistent PSUM bank; L2-norm sums via ACT Square+accum_out.
     After all chunks: scale logits by rsqrt(row)/rsqrt(col) (+temperature),
     per-head softmax on the 5 diagonal block pieces, attn@v + projection.
"""

import os
import sys
import numpy as np

for _p in ("/opt/trn_rl_repo",):
    if _p not in sys.path and os.path.isdir(_p):
        sys.path.insert(0, _p)

import ml_dtypes

BF16 = ml_dtypes.bfloat16

B, C, H, W = 8, 192, 128, 128
HEADS, DH = 4, 48
C3 = 3 * C            # 576
HW = H * W            # 16384
RT = W + 2            # padded row stride = 130 (col0 pad, 1..128 data, 129 pad)
TY = 16               # image rows per chunk
NCHUNK = H // TY      # 8
SLOTS = TY + 2        # 18 row slots per chunk (halo)
# partition tiles over 576 channels
PT5 = [128, 128, 128, 128, 64]
PO5 = [0, 128, 256, 384, 512]
# partition tiles over 192
PT2 = [128, 64]
PO2 = [0, 128]

_CACHE = {}


def _build():
    import concourse.bass as bass
    import concourse.bacc as bacc
    import concourse.tile as tile
    import concourse.mybir as mybir

    f32 = mybir.dt.float32
    bf16 = mybir.dt.bfloat16
    Alu = mybir.AluOpType
    Act = mybir.ActivationFunctionType

    nc = bacc.Bacc("TRN2", target_bir_lowering=False, debug=False,
                   enable_asserts=False)

    x_d = nc.dram_tensor("x0", [C, H, W], bf16, kind="ExternalInput").ap()
    wq_d = nc.dram_tensor("wq", [C, C3], bf16, kind="ExternalInput").ap()
    wdg_d = nc.dram_tensor("wdg", [128, 45 * 128], bf16, kind="ExternalInput").ap()
    wp_d = nc.dram_tensor("wp", [C, C], bf16, kind="ExternalInput").ap()
    tv_d = nc.dram_tensor("tv", [C, 1], f32, kind="ExternalInput").ap()
    idb_d = nc.dram_tensor("idb", [128, 128], bf16, kind="ExternalInput").ap()
    idf_d = nc.dram_tensor("idf", [128, 128], f32, kind="ExternalInput").ap()
    ones_d = nc.dram_tensor("ones1", [1, 128], f32, kind="ExternalInput").ap()
    mka_d = nc.dram_tensor("mka", [128, C], f32, kind="ExternalInput").ap()
    mkb_d = nc.dram_tensor("mkb", [64, C], f32, kind="ExternalInput").ap()
    out_d = nc.dram_tensor("out0", [C, H, W], f32, kind="ExternalOutput").ap()

    with tile.TileContext(nc) as tc:
        with (
            tc.tile_pool(name="const", bufs=1) as cpool,
            tc.tile_pool(name="big", bufs=1) as big,
            tc.tile_pool(name="xin", bufs=2) as xpool,
            tc.tile_pool(name="qk", bufs=1) as qkpool,
            tc.tile_pool(name="qk2", bufs=2) as qkpool2,
            tc.tile_pool(name="small", bufs=1) as small,
        ):
            # ---- constants into SBUF ----
            wq_a = cpool.tile([128, C3], bf16, tag="wqa")
            wq_b = cpool.tile([64, C3], bf16, tag="wqb")
            wdg = cpool.tile([128, 45, 128], bf16, tag="wdg")
            wp_a = cpool.tile([128, C], bf16, tag="wpa")
            wp_b = cpool.tile([64, C], bf16, tag="wpb")
            idb = cpool.tile([128, 128], bf16, tag="idb")
            idf = cpool.tile([128, 128], f32, tag="idf")
            ones1 = cpool.tile([1, 128], f32, tag="ones1")
            tv_a = cpool.tile([128, 1], f32, tag="tva")
            tv_b = cpool.tile([64, 1], f32, tag="tvb")
            # order: gemm1 operands first (wq, then chunk-0 x below), then
            # the depthwise weights, then everything only needed later
            nc.sync.dma_start(wq_a[:], wq_d[0:128, :])
            nc.sync.dma_start(wq_b[:], wq_d[128:192, :])
            # prefetch chunk 0's x slab ahead of the remaining constants
            xa0 = xpool.tile([128, SLOTS, W], bf16, tag="xa")
            xb0 = xpool.tile([64, SLOTS, W], bf16, tag="xb")
            nc.vector.memset(xa0[:, 0, :], 0.0)
            nc.vector.memset(xb0[:, 0, :], 0.0)
            nc.sync.dma_start(xa0[:, 1:SLOTS, :], x_d[0:128, 0:SLOTS - 1, :])
            nc.sync.dma_start(xb0[:, 1:SLOTS, :], x_d[128:192, 0:SLOTS - 1, :])
            nc.sync.dma_start(wdg[:], wdg_d[:, :])
            nc.sync.dma_start(idb[:], idb_d[:])
            nc.sync.dma_start(wp_a[:], wp_d[0:128, :])
            nc.sync.dma_start(wp_b[:], wp_d[128:192, :])
            nc.sync.dma_start(idf[:], idf_d[:])
            nc.sync.dma_start(ones1[:], ones_d[:])
            nc.sync.dma_start(tv_a[:], tv_d[0:128, :])
            nc.sync.dma_start(tv_b[:], tv_d[128:192, :])
            mk_a = cpool.tile([128, C], f32, tag="mka")
            mk_b = cpool.tile([64, C], f32, tag="mkb")
            nc.sync.dma_start(mk_a[:], mka_d[:])
            nc.sync.dma_start(mk_b[:], mkb_d[:])

            # ---- persistent big buffers ----
            # padded qkv (post 1x1 conv), f32: [128p, 5 ch-tiles, 18 slots, 130]
            qkv = big.tile([128, 5, SLOTS, RT], bf16, tag="qkv")
            # only the pad columns (0 and RT-1) need zeroing; the data
            # region is overwritten every chunk and the halo slots get
            # zeros via the zeroed x edge rows.
            nc.vector.memset(qkv[:, :, :, 0:1], 0.0)
            nc.vector.memset(qkv[:, :, :, RT - 1:RT], 0.0)
            # v in bf16 for pass 2: [128p, 128 row-groups of 128, 128]
            # v_b is padded to 128 partitions (upper half zero) so every
            # pass-2 matmul is a uniform full-array K=128 config -- the
            # K=64 half-array alternation defeats LDWEIGHTS pull-ahead.
            v_a = big.tile([128, H, W], bf16, tag="va")
            v_b = big.tile([128, H, W], bf16, tag="vb")
            nc.vector.memset(v_b[64:128, :, :], 0.0)
            # l2 norm partial sums per chunk
            qss_a = small.tile([128, NCHUNK], f32, tag="qssa")
            qss_b = small.tile([64, NCHUNK], f32, tag="qssb")
            kss_a = small.tile([128, NCHUNK], f32, tag="kssa")
            kss_b = small.tile([64, NCHUNK], f32, tag="kssb")
            # (Wp @ attn).T, padded to K=128 like v_b
            MT_a = small.tile([128, C], bf16, tag="MTa")
            MT_b = small.tile([128, C], bf16, tag="MTb")
            nc.vector.memset(MT_b[64:128, :], 0.0)

            with (
                tc.tile_pool(name="psg", bufs=2, space="PSUM") as psg,
                tc.tile_pool(name="psd", bufs=2, space="PSUM") as psd,
                tc.tile_pool(name="pst", bufs=2, space="PSUM") as pst,
                tc.tile_pool(name="psl", bufs=1, space="PSUM") as psl,
            ):
                lg_a = psl.tile([128, C], f32, tag="lga")
                lg_b = psl.tile([64, C], f32, tag="lgb")

                for ch in range(NCHUNK):
                    y0 = ch * TY
                    # ---- load x chunk (rows y0-1 .. y0+16) ----
                    if ch == 0:
                        xa, xb = xa0, xb0
                    else:
                        xa = xpool.tile([128, SLOTS, W], bf16, tag="xa")
                        xb = xpool.tile([64, SLOTS, W], bf16, tag="xb")
                        lo = y0 - 1
                        hi = min(y0 + TY, H - 1)
                        s1 = hi - (y0 - 1)
                        if ch == NCHUNK - 1:
                            nc.vector.memset(xa[:, SLOTS - 1, :], 0.0)
                            nc.vector.memset(xb[:, SLOTS - 1, :], 0.0)
                        nc.sync.dma_start(xa[:, 0:s1 + 1, :], x_d[0:128, lo:hi + 1, :])
                        nc.sync.dma_start(xb[:, 0:s1 + 1, :], x_d[128:192, lo:hi + 1, :])

                    # ---- GEMM1: qkv = wq.T @ x, 3 rows (384 cols) at a time ----
                    for r in range(5):
                        po, pn = PO5[r], PT5[r]
                        for j in range(SLOTS // 3):
                            pg = psg.tile([128, 3, W], f32, tag="pg")
                            nc.tensor.matmul(
                                pg[0:pn, :, :], wq_a[:, po:po + pn],
                                xa[:, 3 * j:3 * j + 3, :], start=True, stop=False)
                            nc.tensor.matmul(
                                pg[0:pn, :, :], wq_b[:, po:po + pn],
                                xb[:, 3 * j:3 * j + 3, :], start=False, stop=True)
                            nc.any.tensor_copy(
                                qkv[0:pn, r, 3 * j:3 * j + 3, 1:1 + W],
                                pg[0:pn, :, :])

                    # ---- depthwise 3x3 + split/copy q,k,v, interleaved with
                    # the q/k transposes and logit matmuls so the PE always
                    # has HAM-visible (regular matmul) work in flight ----
                    q_a = qkpool2.tile([128, TY, W], bf16, tag="qa")
                    q_b = qkpool2.tile([64, TY, W], bf16, tag="qb")
                    k_a = qkpool2.tile([128, TY, W], bf16, tag="ka")
                    k_b = qkpool2.tile([64, TY, W], bf16, tag="kb")
                    qT = qkpool.tile([128, TY, C], bf16, tag="qT")
                    kT = qkpool.tile([128, TY, C], bf16, tag="kT")

                    def dw(r, t):
                        po, pn = PO5[r], PT5[r]
                        pd = psd.tile([128, 4, W], f32, tag="pd")
                        for kk in range(9):
                            dy, dx = kk // 3 - 1, kk % 3 - 1
                            srow = 1 + 4 * t + dy
                            nc.tensor.matmul(
                                pd[0:pn, :, :],
                                wdg[0:pn, r * 9 + kk, 0:pn],
                                qkv[0:pn, r, srow:srow + 4, 1 + dx:1 + dx + W],
                                start=(kk == 0), stop=(kk == 8))
                        dst = 4 * t
                        if r == 0:
                            nc.any.tensor_copy(q_a[:, dst:dst + 4, :], pd[:, :, :])
                        elif r == 1:
                            nc.any.tensor_copy(q_b[0:64, dst:dst + 4, :], pd[0:64, :, :])
                            nc.any.tensor_copy(k_a[0:64, dst:dst + 4, :], pd[64:128, :, :])
                        elif r == 2:
                            nc.any.tensor_copy(k_a[64:128, dst:dst + 4, :], pd[0:64, :, :])
                            nc.any.tensor_copy(k_b[0:64, dst:dst + 4, :], pd[64:128, :, :])
                        elif r == 3:
                            nc.any.tensor_copy(v_a[:, y0 + dst:y0 + dst + 4, :], pd[:, :, :])
                        else:
                            nc.any.tensor_copy(v_b[0:64, y0 + dst:y0 + dst + 4, :], pd[0:64, :, :])

                    def tr(s):
                        # transpose one slot of q and k into qT/kT (PSUM hop)
                        tpq = pst.tile([128, C], bf16, tag="tp")
                        nc.tensor.transpose(tpq[:, 0:128], q_a[:, s, :], idb[:])
                        nc.tensor.transpose(tpq[:, 128:192], q_b[:, s, :], idb[0:64, 0:64])
                        nc.any.tensor_copy(qT[:, s, :], tpq[:, :])
                        tpk = pst.tile([128, C], bf16, tag="tp")
                        nc.tensor.transpose(tpk[:, 0:128], k_a[:, s, :], idb[:])
                        nc.tensor.transpose(tpk[:, 128:192], k_b[:, s, :], idb[0:64, 0:64])
                        nc.any.tensor_copy(kT[:, s, :], tpk[:, :])

                    def lg(s):
                        first = (ch == 0 and s == 0)
                        last = (ch == NCHUNK - 1 and s == TY - 1)
                        nc.tensor.matmul(lg_a[:, :], qT[:, s, 0:128], kT[:, s, :],
                                         start=first, stop=last, skip_group_check=True)
                        nc.tensor.matmul(lg_b[:, :], qT[:, s, 128:192], kT[:, s, :],
                                         start=first, stop=last, skip_group_check=True)

                    # interleave so no PE transpose bunch exceeds ~1us of
                    # HAM-invisible work between regular matmul stretches
                    for t in range(TY // 4):
                        dw(0, t)
                        dw(1, t)
                        dw(2, t)
                        tr(4 * t)
                        dw(3, t)
                        tr(4 * t + 1)
                        dw(4, t)
                        tr(4 * t + 2)
                        lg(4 * t)
                        tr(4 * t + 3)
                        lg(4 * t + 1)
                        lg(4 * t + 2)
                        lg(4 * t + 3)

                    # ---- L2 norm partial sums (ACT: square + accum) ----
                    sqs = qkpool.tile([128, TY, W], bf16, tag="sqs")
                    nc.scalar.activation(sqs[:, :, :], q_a[:, :, :], Act.Square,
                                         accum_out=qss_a[:, ch:ch + 1])
                    nc.scalar.activation(sqs[0:64, :, :], q_b[:, :, :], Act.Square,
                                         accum_out=qss_b[:, ch:ch + 1])
                    nc.scalar.activation(sqs[:, :, :], k_a[:, :, :], Act.Square,
                                         accum_out=kss_a[:, ch:ch + 1])
                    nc.scalar.activation(sqs[0:64, :, :], k_b[:, :, :], Act.Square,
                                         accum_out=kss_b[:, ch:ch + 1])

                # ================= softmax =================
                # rsqrt of row sums (+ temperature on q side)
                rq_a = small.tile([128, 1], f32, tag="rqa")
                rq_b = small.tile([64, 1], f32, tag="rqb")
                rk_a = small.tile([128, 1], f32, tag="rka")
                rk_b = small.tile([64, 1], f32, tag="rkb")
                tmp_a = small.tile([128, 1], f32, tag="tmpa")
                tmp_b = small.tile([64, 1], f32, tag="tmpb")
                for (ss, rr, tmp, tvx) in ((qss_a, rq_a, tmp_a, tv_a),
                                           (qss_b, rq_b, tmp_b, tv_b),
                                           (kss_a, rk_a, tmp_a, None),
                                           (kss_b, rk_b, tmp_b, None)):
                    nc.vector.tensor_reduce(tmp[:], ss[:], mybir.AxisListType.X, Alu.add)
                    nc.scalar.activation(tmp[:], tmp[:], Act.Sqrt)
                    nc.vector.tensor_scalar_max(tmp[:], tmp[:], 1e-12)
                    nc.vector.reciprocal(rr[:], tmp[:])
                    if tvx is not None:
                        nc.vector.tensor_tensor(rr[:], rr[:], tvx[:], Alu.mult)

                # copy logits out of psum, scale rows by rq
                L_a = small.tile([128, C], f32, tag="La")
                L_b = small.tile([64, C], f32, tag="Lb")
                nc.vector.tensor_scalar(L_a[:], lg_a[:], rq_a[:], None, Alu.mult)
                nc.vector.tensor_scalar(L_b[:], lg_b[:], rq_b[:], None, Alu.mult)

            with tc.tile_pool(name="psx", bufs=1, space="PSUM") as psx:
                # warm-keeper: dummy matmuls that read the freshly-written
                # softmax intermediates.  Each is data-dependent on the
                # producing op, so they spread across the softmax phase and
                # keep the PE clock-gate (HAM) from re-throttling before
                # pass 2 starts.
                warm = psx.tile([128, C], f32, tag="warm")

                def keep_warm(src, np_=128):
                    nc.tensor.matmul(warm[0:np_, :], idf[0:np_, 0:np_], src,
                                     start=True, stop=True)

                # column scale: bcast rk over partitions via K=1 matmul
                rkrow = small.tile([1, C], f32, tag="rkrow")
                pb = psx.tile([128, C], f32, tag="pb")
                nc.tensor.transpose(pb[0:1, 0:128], rk_a[:], idf[:])
                nc.tensor.transpose(pb[0:1, 128:192], rk_b[:], idf[0:64, 0:64])
                nc.any.tensor_copy(rkrow[:], pb[0:1, 0:192])
                pbc = psx.tile([128, C], f32, tag="pbc")
                nc.tensor.matmul(pbc[:, :], ones1[:], rkrow[:], start=True, stop=True)
                keep_warm(L_a[:, :])
                nc.vector.tensor_tensor(L_a[:], L_a[:], pbc[:, :], Alu.mult)
                nc.vector.tensor_tensor(L_b[:], L_b[:], pbc[0:64, :], Alu.mult)
                keep_warm(L_a[:, :])

                # full-row softmax; cross-head blocks masked to -1e30 -> exp 0
                attn_a = small.tile([128, C], bf16, tag="atta")
                attn_b = small.tile([64, C], bf16, tag="attb")
                mx = small.tile([128, 1], f32, tag="mx")
                sm = small.tile([128, 1], f32, tag="sm")
                E = small.tile([128, C], f32, tag="E")
                for (L, A, mk, np_) in ((L_a, attn_a, mk_a, 128),
                                        (L_b, attn_b, mk_b, 64)):
                    nc.vector.tensor_tensor(L[:], L[:], mk[:], Alu.add)
                    keep_warm(L[:, :], np_)
                    nc.vector.tensor_reduce(mx[0:np_, :], L[:],
                                            mybir.AxisListType.X, Alu.max)
                    nc.vector.tensor_scalar_mul(mx[0:np_, :], mx[0:np_, :], -1.0)
                    nc.scalar.activation(E[0:np_, :], L[:], Act.Exp,
                                         bias=mx[0:np_, :],
                                         accum_out=sm[0:np_, :])
                    keep_warm(E[0:np_, :], np_)
                    nc.vector.reciprocal(sm[0:np_, :], sm[0:np_, :])
                    nc.scalar.activation(A[:], E[0:np_, :], Act.Copy,
                                         scale=sm[0:np_, :])
                    keep_warm(E[0:np_, :], np_)

                # MT[d, o] = sum_c attn[c, d] * wp[c, o]  ((Wp @ attn)
                # transposed) -- folds the projection into a tiny GEMM so
                # pass 2 is a single stream over v.
                pMa = psx.tile([128, C], f32, tag="pMa")
                pMb = psx.tile([64, C], f32, tag="pMb")
                nc.tensor.matmul(pMa[:, :], attn_a[:, 0:128], wp_a[:, :],
                                 start=True, stop=False)
                nc.tensor.matmul(pMa[:, :], attn_b[:, 0:128], wp_b[:, :],
                                 start=False, stop=True)
                nc.tensor.matmul(pMb[:, :], attn_a[:, 128:192], wp_a[:, :],
                                 start=True, stop=False)
                nc.tensor.matmul(pMb[:, :], attn_b[:, 128:192], wp_b[:, :],
                                 start=False, stop=True)
                nc.any.tensor_copy(MT_a[:, :], pMa[:, :])
                nc.any.tensor_copy(MT_b[0:64, :], pMb[:, :])

            # ---- pass 2: out = MT.T @ v (uniform K=128 matmuls) ----
            with (
                tc.tile_pool(name="ps2", bufs=4, space="PSUM") as ps2,
                tc.tile_pool(name="o2", bufs=3) as opool,
            ):
                    for gp in range(HW // 1024):
                        ot_a = opool.tile([128, 8, W], f32, tag="ota")
                        ot_b = opool.tile([64, 8, W], f32, tag="otb")
                        for hh in range(2):
                            g = 2 * gp + hh
                            r4 = 4 * g
                            poa = ps2.tile([128, 4, W], f32, tag="poa")
                            pob = ps2.tile([64, 4, W], f32, tag="pob")
                            nc.tensor.matmul(poa[:, :, :], MT_a[:, 0:128],
                                             v_a[:, r4:r4 + 4, :], start=True, stop=False)
                            nc.tensor.matmul(pob[:, :, :], MT_a[:, 128:192],
                                             v_a[:, r4:r4 + 4, :], start=True, stop=False)
                            nc.tensor.matmul(poa[:, :, :], MT_b[:, 0:128],
                                             v_b[:, r4:r4 + 4, :], start=False, stop=True)
                            nc.tensor.matmul(pob[:, :, :], MT_b[:, 128:192],
                                             v_b[:, r4:r4 + 4, :], start=False, stop=True)
                            nc.vector.tensor_copy(ot_a[:, 4 * hh:4 * hh + 4, :],
                                                  poa[:, :, :])
                            nc.any.tensor_copy(ot_b[:, 4 * hh:4 * hh + 4, :],
                                               pob[:, :, :])
                        r8 = 8 * gp
                        nc.sync.dma_start(out_d[0:128, r8:r8 + 8, :], ot_a[:])
                        nc.sync.dma_start(out_d[128:192, r8:r8 + 8, :], ot_b[:])

    nc.compile()
    return nc


def _prep_weights(w_qkv, w_dw, w_project, temperature):
    wq = np.ascontiguousarray(w_qkv.T).astype(BF16)              # [192, 576]
    wp = np.ascontiguousarray(w_project.T).astype(BF16)          # [192, 192]
    # diagonal stationaries: [128, 45*128] f32, block (r*9+k) = diag(w_dw[ch, k])
    wdg = np.zeros((128, 45, 128), np.float32)
    wd = w_dw.reshape(C3, 9)
    for r in range(5):
        po, pn = PO5[r], PT5[r]
        for k in range(9):
            blk = wdg[:, r * 9 + k, :]
            blk[np.arange(pn), np.arange(pn)] = wd[po:po + pn, k]
    tv = np.repeat(temperature.reshape(HEADS), DH).reshape(C, 1).astype(np.float32)
    mk = np.full((C, C), -1e30, np.float32)
    for h in range(HEADS):
        mk[h * DH:(h + 1) * DH, h * DH:(h + 1) * DH] = 0.0
    return {
        "wq": wq,
        "wp": wp,
        "wdg": wdg.reshape(128, 45 * 128).astype(BF16),
        "tv": tv,
        "idb": np.eye(128, dtype=BF16),
        "idf": np.eye(128, dtype=np.float32),
        "ones1": np.ones((1, 128), np.float32),
        "mka": mk[0:128],
        "mkb": mk[128:192],
    }


def kernel(x, w_qkv, w_dw, w_project, temperature, heads):
    from concourse import bass_utils

    x = np.asarray(x, np.float32)
    key = "nc"
    if key not in _CACHE:
        _CACHE[key] = _build()
    nc = _CACHE[key]

    shared = _prep_weights(np.asarray(w_qkv, np.float32),
                           np.asarray(w_dw, np.float32),
                           np.asarray(w_project, np.float32),
                           np.asarray(temperature, np.float32))
    in_maps = []
    for i in range(B):
        m = dict(shared)
        m["x0"] = x[i].reshape(C, H, W).astype(BF16)
        in_maps.append(m)

    res = bass_utils.run_bass_kernel_spmd(nc, in_maps, core_ids=list(range(B)))
    outs = [r["out0"].reshape(C, H, W) for r in res.results]
    return np.stack(outs, axis=0).astype(np.float32)


if __name__ == "__main__":
    rng = np.random.default_rng(0)
    x = rng.standard_normal((B, C, H, W), np.float32)
    w_qkv = rng.standard_normal((C3, C), np.float32) / np.sqrt(C)
    w_dw = rng.standard_normal((C3, 1, 3, 3), np.float32) / 3.0
    w_project = rng.standard_normal((C, C), np.float32) / np.sqrt(C)
    temperature = np.ones((HEADS, 1, 1), np.float32)
    y = kernel(x=x, w_qkv=w_qkv, w_dw=w_dw, w_project=w_project,
               temperature=temperature, heads=HEADS)
    print(y.shape, y.dtype)

